# revision 34
# baseline (speedup 1.0000x reference)
"""Trainium2 Bass kernel for nn_Attention_23802708754880 (sparse_attention).

v3: AllToAll head-resharding. Each core projects Q/K/V/gates for its OWN
512 rows (b-concat, sequence-parallel), then an AllToAll hands head-owner
core d the rows of ALL cores for kv-head d (+ its 2 GQA q-heads + gates).
Attention, belief and gating run head-sharded over the full sequence; a
second (per-batch) AllToAll returns finished head outputs to row-owners,
which run the output projection. AllToAll moves 8x less data than the
old AllGather (each element goes to exactly one core): ~2.1MB + 2x0.5MB
vs 16.9MB gathered, so the serialized-collective wall shrinks ~240->~120us.

Rank symmetry: slot d of the a2a input = data for head d; output slot s =
data from core s. No rank-dependent indexing anywhere.

Numerics: tanh softclamp skipped (|logits|<=6), K/V/Q/P/Wo bf16,
denominator fused via a ones-column per head block (65-col V slabs).
"""
import os
import numpy as np

import concourse.bass as bass
import concourse.tile as tile
from concourse import bacc, mybir
from concourse.bass_utils import run_bass_kernel_spmd

FP = mybir.dt.float32
BF = mybir.dt.bfloat16
N_CORES = 8
B = 2
N = 2048
D = 2048
DH = 64
H = 8
QH = 16
RB = N // N_CORES          # 256
R = B * RB                 # 512
NKT = D // 128             # 16
EPS_RMS = float(np.finfo(np.float32).eps)
SCALE = DH ** -0.5         # 0.125; exp(scale * s_raw) directly (tanh skipped)

# a2a1 is split: part A carries Q^T [64,1024] + K^T [64,512] per chunk (so the
# Act-bound sim/exp work can start as soon as it lands); part B carries
# V 4x[128,65] + gates [2,512].
OFF_Q = 0
OFF_K = 65536
C1A = 98304
VSLAB = 128 * 65           # 8320
OFF_G = 4 * VSLAB          # 33280
C1B = OFF_G + 1024         # 34304

_cache = {}


def build_kernel(iters=1):
    nc = bacc.Bacc("TRN2", target_bir_lowering=False, debug=False, num_devices=N_CORES)

    x_in = nc.dram_tensor("x_loc", [R, D], FP, kind="ExternalInput")
    wq_in = nc.dram_tensor("wq", [D, QH * DH], mybir.dt.float32r, kind="ExternalInput")
    wk_in = nc.dram_tensor("wk", [D, H * DH], mybir.dt.float32r, kind="ExternalInput")
    wcat_in = nc.dram_tensor("wcat", [D, 536], mybir.dt.float32r, kind="ExternalInput")
    wo_in = nc.dram_tensor("wo_bf", [QH * DH, D], BF, kind="ExternalInput")
    res_in = nc.dram_tensor("res_pp", [4, 128, 512], FP, kind="ExternalInput")
    cos_in = nc.dram_tensor("cosT", [128, R], FP, kind="ExternalInput")
    sin_in = nc.dram_tensor("sinTs", [128, R], FP, kind="ExternalInput")
    gsc_in = nc.dram_tensor("gamma_sc", [128, 4], FP, kind="ExternalInput")
    gscp_in = nc.dram_tensor("gamma_scp", [128, 4], FP, kind="ExternalInput")  # partner-permuted
    bmix_in = nc.dram_tensor("bmix_t", [128, H], FP, kind="ExternalInput")
    ident_in = nc.dram_tensor("ident", [128, 128], FP, kind="ExternalInput")
    bd2_in = nc.dram_tensor("bd2", [128, 2], FP, kind="ExternalInput")
    bd2T_in = nc.dram_tensor("bd2T", [2, 128], FP, kind="ExternalInput")
    ones128_in = nc.dram_tensor("ones128", [128, 1], FP, kind="ExternalInput")
    ones128r_in = nc.dram_tensor("ones128r", [128, 32], mybir.dt.float32r, kind="ExternalInput")
    ones1_in = nc.dram_tensor("ones1", [1, 64], FP, kind="ExternalInput")
    ones1r_in = nc.dram_tensor("ones1r", [1, 64], mybir.dt.float32r, kind="ExternalInput")
    out_dram = nc.dram_tensor("out_loc", [R, D], FP, kind="ExternalOutput")

    Exp = mybir.ActivationFunctionType.Exp
    Sqrt = mybir.ActivationFunctionType.Sqrt
    Sigmoid = mybir.ActivationFunctionType.Sigmoid
    Copy = mybir.ActivationFunctionType.Copy
    Square = mybir.ActivationFunctionType.Square
    ADD = mybir.AluOpType.add
    MULT = mybir.AluOpType.mult

    FR = mybir.dt.float32r

    def MM(out, lhsT, rhs, **kw):
        return nc.tensor.matmul(out, lhsT, rhs, **kw)

    import contextlib
    with tile.TileContext(nc) as tc, contextlib.ExitStack() as ctx:
        consts = ctx.enter_context(tc.tile_pool(name="consts", bufs=1))
        xpool = ctx.enter_context(tc.tile_pool(name="xpool", bufs=1))
        ntpool = ctx.enter_context(tc.tile_pool(name="ntpool", bufs=1))
        wstream = ctx.enter_context(tc.tile_pool(name="wstream", bufs=1))
        qtpool = ctx.enter_context(tc.tile_pool(name="qtpool", bufs=1))
        kvloc = ctx.enter_context(tc.tile_pool(name="kvloc", bufs=1))
        gath = ctx.enter_context(tc.tile_pool(name="gath", bufs=1))
        scr = ctx.enter_context(tc.tile_pool(name="scr", bufs=1))
        smalls = ctx.enter_context(tc.tile_pool(name="smalls", bufs=1))
        simsb = ctx.enter_context(tc.tile_pool(name="simsb", bufs=1))
        finpool = ctx.enter_context(tc.tile_pool(name="finpool", bufs=1))
        ps = ctx.enter_context(tc.tile_pool(name="ps", bufs=1, space="PSUM"))
        dram = ctx.enter_context(tc.tile_pool(name="dram", bufs=1, space="DRAM"))

        # ---------------- constants ----------------
        ident = consts.tile([128, 128], FP, bufs=1)
        cosT = consts.tile([128, R], FP, bufs=1)
        sinTs = consts.tile([128, R], FP, bufs=1)
        gsc = consts.tile([128, 4], FP, bufs=1)
        gscp = consts.tile([128, 4], FP, bufs=1)
        bmixt = consts.tile([128, H], FP, bufs=1)
        bd2 = consts.tile([128, 2], FP, bufs=1)
        bd2T = consts.tile([2, 128], FP, bufs=1)
        ones128 = consts.tile([128, 1], FP, bufs=1)
        ones128r = consts.tile([128, 32], mybir.dt.float32r, bufs=1)
        ones1 = consts.tile([1, 64], FP, bufs=1)
        ones1r = consts.tile([1, 64], mybir.dt.float32r, bufs=1)
        nc.sync.dma_start(out=ident, in_=ident_in[:, :])
        nc.sync.dma_start(out=cosT, in_=cos_in[:, :])
        nc.sync.dma_start(out=sinTs, in_=sin_in[:, :])
        nc.sync.dma_start(out=gsc, in_=gsc_in[:, :])
        nc.sync.dma_start(out=gscp, in_=gscp_in[:, :])
        nc.sync.dma_start(out=bmixt, in_=bmix_in[:, :])
        nc.sync.dma_start(out=bd2, in_=bd2_in[:, :])
        nc.sync.dma_start(out=bd2T, in_=bd2T_in[:, :])
        nc.sync.dma_start(out=ones128, in_=ones128_in[:, :])
        nc.sync.dma_start(out=ones128r, in_=ones128r_in[:, :])
        nc.sync.dma_start(out=ones1, in_=ones1_in[:, :])
        nc.sync.dma_start(out=ones1r, in_=ones1r_in[:, :])
        eps_rms = consts.tile([128, 1], FP, bufs=1)
        nc.vector.memset(eps_rms, EPS_RMS)
        eps24 = consts.tile([128, 1], FP, bufs=1)
        nc.vector.memset(eps24, 1e-24)

        for it in range(iters):
            # ---------------- load raw x + transpose -> nt (no rmsnorm) ------
            xts = []
            for rt in range(4):
                xt = xpool.tile([128, D], FP, name=f"i{it}_xt{rt}", tag="xt", bufs=2)
                nc.sync.dma_start(out=xt, in_=x_in[rt * 128:(rt + 1) * 128, :])
                xts.append(xt)
            nt = [ntpool.tile([128, R], FR, name=f"i{it}_nT{ct}", tag=f"nt{ct}", bufs=1)
                  for ct in range(NKT)]
            for rt in range(4):
                for ct in range(NKT):
                    tp = ps.tile([128, 128], FP, name=f"i{it}_trp{rt}_{ct}", tag="bsmall", bufs=2)
                    nc.tensor.transpose(tp, xts[rt][:, ct * 128:(ct + 1) * 128], ident)
                    if ct % 2 == 0:
                        nc.vector.tensor_copy(nt[ct][:, rt * 128:(rt + 1) * 128], tp)
                    else:
                        nc.scalar.activation(out=nt[ct][:, rt * 128:(rt + 1) * 128],
                                             in_=tp, func=Copy)

            # rmsnorm factor r per local row; squares xt in place (dead after
            # the transposes) with an accumulate output.
            rnat = []
            for rt in range(4):
                ssum = smalls.tile([128, 1], FP, name=f"i{it}_ssum{rt}", tag="ssum", bufs=2)
                nc.scalar.activation(out=xts[rt], in_=xts[rt], func=Square, accum_out=ssum)
                sd = smalls.tile([128, 1], FP, name=f"i{it}_sd{rt}", tag=f"sd{rt}", bufs=1)
                nc.scalar.activation(out=sd, in_=ssum, func=Sqrt, scale=1.0 / D, bias=eps_rms[:, 0:1])
                nc.vector.reciprocal(out=sd, in_=sd)
                rnat.append(sd)

            # ---------------- K^T projection, l2norm*gamma, rotary (no r) ----
            kt_ts = []
            wk_r = wk_in.rearrange("(a p) c -> p a c", p=128)  # (128, 16, 512)
            for t in range(4):
                kp = ps.tile([128, R], FP, name=f"i{it}_kp{t}", tag="acc", bufs=2)
                for hf in range(2):
                    wkm = wstream.tile([128, NKT // 2, 128], FR, name=f"i{it}_wkm{t}_{hf}", tag="wstream", bufs=2)
                    nc.sync.dma_start(out=wkm, in_=wk_r[:, hf * 8:(hf + 1) * 8, t * 128:(t + 1) * 128])
                    for k8 in range(8):
                        kt = hf * 8 + k8
                        MM(kp, wkm[:, k8, :], nt[kt],
                                         start=(kt == 0), stop=(kt == NKT - 1))
                kpsb = scr.tile([128, R], FP, name=f"i{it}_kpsb{t}", tag="SC", bufs=2)
                nc.scalar.activation(out=kpsb, in_=kp, func=Copy)
                ksq = scr.tile([128, R], FP, name=f"i{it}_ksq{t}", tag="SA", bufs=1)
                nc.scalar.activation(out=ksq, in_=kp, func=Square)
                nrm = ps.tile([2, R], FP, name=f"i{it}_nrm{t}", tag="bsmall", bufs=2)
                MM(nrm, bd2, ksq, start=True, stop=True)
                sdk = smalls.tile([2, R], FP, name=f"i{it}_sdk{t}", tag="sdk", bufs=2)
                nc.scalar.activation(out=sdk, in_=nrm, func=Sqrt, bias=eps24[0:2, 0:1])
                nc.vector.reciprocal(out=sdk, in_=sdk)
                bn = ps.tile([128, R], FP, name=f"i{it}_bn{t}", tag="avp", bufs=2)
                MM(bn, bd2T, sdk, start=True, stop=True)
                k1 = scr.tile([128, R], FP, name=f"i{it}_k1_{t}", tag="SB", bufs=1)
                nc.vector.scalar_tensor_tensor(k1, kpsb, gsc[:, t:t + 1], bn, MULT, MULT)
                ta = scr.tile([128, R], FP, name=f"i{it}_kta{t}", tag="SA", bufs=1)
                nc.vector.tensor_mul(ta, k1, cosT)
                tb = scr.tile([128, R], FP, name=f"i{it}_ktb{t}", tag="SD", bufs=1)
                for blk in range(4):
                    pb = blk ^ 1
                    nc.vector.tensor_mul(tb[blk * 32:(blk + 1) * 32, :],
                                         kp[pb * 32:(pb + 1) * 32, :],
                                         sinTs[blk * 32:(blk + 1) * 32, :])
                nc.vector.scalar_tensor_tensor(tb, tb, gscp[:, t:t + 1], bn, MULT, MULT)
                kt_t = kvloc.tile([128, R], BF, name=f"i{it}_KTt{t}", tag=f"KT{t}", bufs=1)
                nc.vector.tensor_add(kt_t, ta, tb)
                kt_ts.append(kt_t)

            # ---------------- V / mix / gates + r + lerp ---------------------
            gates_nat = []
            wcat_r = wcat_in
            accA = ps.tile([128, 1024], FP, name=f"i{it}_vaccA", tag="acc", bufs=2)
            accB = ps.tile([128, 1024], FP, name=f"i{it}_vaccB", tag="acc", bufs=2)
            vps = [accA[:, 0:512], accA[:, 512:1024], accB[:, 0:512], accB[:, 512:1024]]
            vp2s = [ps.tile([128, 24], FP, name=f"i{it}_vp2_{rt}", tag=("bsmall" if rt < 2 else "avp"), bufs=2)
                    for rt in range(4)]
            for kt in range(NKT):
                wc = wstream.tile([128, 536], FR, name=f"i{it}_wc{kt}", tag="wc", bufs=2)
                nc.sync.dma_start(out=wc, in_=wcat_r[kt * 128:(kt + 1) * 128, :])
                for rt in range(4):
                    lhsT = nt[kt][:, rt * 128:(rt + 1) * 128]
                    MM(vps[rt], lhsT, wc[:, 0:512],
                                     start=(kt == 0), stop=(kt == NKT - 1))
                    MM(vp2s[rt], lhsT, wc[:, 512:536],
                                     start=(kt == 0), stop=(kt == NKT - 1))
            vsbs = []
            for rt in range(4):
                vp = vps[rt]
                vp2 = vp2s[rt]
                rs = xpool.tile([128, 512], FP, name=f"i{it}_rs{rt}", tag="rs", bufs=2)
                nc.sync.dma_start(out=rs, in_=res_in[rt, :, :])
                vp2r = smalls.tile([128, 24], FP, name=f"i{it}_vp2r{rt}", tag=f"vp2r{rt}", bufs=1)
                nc.vector.tensor_scalar_mul(vp2r, vp2, rnat[rt])
                mixl = smalls.tile([128, H], FP, name=f"i{it}_mixl{rt}", tag=f"mixl{rt}", bufs=1)
                nc.vector.tensor_add(mixl, vp2r[:, 0:8], bmixt)
                nc.scalar.activation(out=mixl, in_=mixl, func=Sigmoid)
                gn = smalls.tile([128, QH], FP, name=f"i{it}_gn{rt}", tag=f"gn{rt}", bufs=1)
                nc.scalar.activation(out=gn, in_=vp2r[:, 8:24], func=Sigmoid)
                gates_nat.append(gn)
                vr = scr.tile([128, 512], FP, name=f"i{it}_vr{rt}", tag="SC", bufs=2)
                nc.vector.tensor_scalar_mul(vr, vp, rnat[rt])
                d1 = scr.tile([128, 512], FP, name=f"i{it}_d1_{rt}", tag="SA", bufs=1)
                nc.vector.tensor_sub(d1, rs, vr)
                d2 = scr.tile([128, 512], FP, name=f"i{it}_d2_{rt}", tag="SB", bufs=1)
                mix_b = mixl[:, :].unsqueeze(-1).to_broadcast([128, H, DH])
                nc.vector.tensor_mul(d2.rearrange("p (h d) -> p h d", d=DH),
                                     d1.rearrange("p (h d) -> p h d", d=DH), mix_b)
                # vsb: [128, 520] bf16, 65-col head blocks, col 64 of each = 1.0
                vsb = kvloc.tile([128, H * 65], BF, name=f"i{it}_vsb{rt}", tag=f"vsb{rt}", bufs=1)
                vsb_h = vsb.rearrange("p (h e) -> p h e", e=65)
                nc.vector.tensor_add(vsb_h[:, :, 0:64],
                                     vr.rearrange("p (h d) -> p h d", d=DH),
                                     d2.rearrange("p (h d) -> p h d", d=DH))
                nc.vector.memset(vsb_h[:, :, 64:65], 1.0)
                vsbs.append(vsb)

            # ---------------- Q^T projection + rotary (r folded) -------------
            rT = smalls.tile([1, R], FP, name=f"i{it}_rT", tag="rT", bufs=1)
            for rt in range(4):
                rtp = ps.tile([1, 128], FP, name=f"i{it}_rtp{rt}", tag="bsmall", bufs=2)
                nc.tensor.transpose(rtp, rnat[rt][:, 0:1], ident)
                nc.vector.tensor_copy(rT[:, rt * 128:(rt + 1) * 128], rtp)
            onesTp = ps.tile([1, 128], FP, name=f"i{it}_onesTp", tag="bsmall", bufs=2)
            nc.tensor.transpose(onesTp, ones128[:, 0:1], ident)
            onesT = smalls.tile([1, 128], FP, name=f"i{it}_onesT", tag="onesT", bufs=1)
            nc.vector.tensor_copy(onesT, onesTp)
            rbcp = ps.tile([128, R], FP, name=f"i{it}_rbcp", tag="avp", bufs=2)
            MM(rbcp, onesT, rT, start=True, stop=True)
            cosR = consts.tile([128, R], FP, bufs=1, name=f"i{it}_cosR", tag="cosR")
            sinR = consts.tile([128, R], FP, bufs=1, name=f"i{it}_sinR", tag="sinR")
            nc.vector.tensor_mul(cosR, cosT, rbcp)
            nc.vector.tensor_mul(sinR, sinTs, rbcp)

            Qpk = [qtpool.tile([128, 2 * R], BF, name=f"i{it}_Qpk{j}", tag=f"Qpk{j}", bufs=1)
                   for j in range(4)]
            wq_r = wq_in.rearrange("(a p) c -> p a c", p=128)  # (128, 16, 1024)
            for m in range(H):
                qp = ps.tile([128, R], FP, name=f"i{it}_qp{m}", tag="acc", bufs=2)
                for hf in range(2):
                    wqm = wstream.tile([128, NKT // 2, 128], FR, name=f"i{it}_wqm{m}_{hf}", tag="wstream", bufs=2)
                    nc.sync.dma_start(out=wqm, in_=wq_r[:, hf * 8:(hf + 1) * 8, m * 128:(m + 1) * 128])
                    for k8 in range(8):
                        kt = hf * 8 + k8
                        MM(qp, wqm[:, k8, :], nt[kt],
                                         start=(kt == 0), stop=(kt == NKT - 1))
                ta = scr.tile([128, R], FP, name=f"i{it}_qta{m}", tag="SA", bufs=1)
                nc.vector.tensor_mul(ta, qp, cosR)
                tb = scr.tile([128, R], FP, name=f"i{it}_qtb{m}", tag="SB", bufs=1)
                for blk in range(4):
                    pb = blk ^ 1
                    nc.vector.tensor_mul(tb[blk * 32:(blk + 1) * 32, :],
                                         qp[pb * 32:(pb + 1) * 32, :],
                                         sinR[blk * 32:(blk + 1) * 32, :])
                hb = 64 * (m % 2)
                nc.vector.tensor_add(Qpk[m // 2][hb:hb + 64, 0:R], ta[0:64, :], tb[0:64, :])
                nc.vector.tensor_add(Qpk[m // 2][hb:hb + 64, R:2 * R], ta[64:128, :], tb[64:128, :])

            # gates transposed: [16 qh, R] bf16
            gatesT = consts.tile([QH, R], BF, bufs=1, name=f"i{it}_gatesT", tag="gatesT")
            for rt in range(4):
                tp = ps.tile([16, 128], FP, name=f"i{it}_gtp{rt}", tag="bsmall", bufs=2)
                nc.tensor.transpose(tp, gates_nat[rt], ident)
                nc.vector.tensor_copy(gatesT[:, rt * 128:(rt + 1) * 128], tp)

            # ---------------- pack + AllToAll #1 -----------------------------
            cc1_in = dram.tile([N_CORES, C1], BF, name=f"i{it}_cc1_in", tag="cc1_in", bufs=2)
            for d in range(N_CORES):
                hb = 64 * (d % 2)
                nc.sync.dma_start(
                    out=cc1_in[d, OFF_Q:OFF_Q + 65536].rearrange("(p f) -> p f", p=64),
                    in_=Qpk[d // 2][hb:hb + 64, 0:2 * R])
                nc.sync.dma_start(
                    out=cc1_in[d, OFF_K:OFF_K + 32768].rearrange("(p f) -> p f", p=64),
                    in_=kt_ts[d // 2][hb:hb + 64, :])
                for j in range(4):
                    nc.sync.dma_start(
                        out=cc1_in[d, OFF_V + j * VSLAB:OFF_V + (j + 1) * VSLAB].rearrange(
                            "(p f) -> p f", p=128),
                        in_=vsbs[j][:, d * 65:(d + 1) * 65])
                nc.sync.dma_start(
                    out=cc1_in[d, OFF_G:OFF_G + 1024].rearrange("(p f) -> p f", p=2),
                    in_=gatesT[2 * d:2 * d + 2, :])
            cc1_out = dram.tile([N_CORES, C1], BF, name=f"i{it}_cc1_out", tag="cc1_out", bufs=2)
            nc.gpsimd.collective_compute(
                "AllToAll", mybir.AluOpType.bypass,
                replica_groups=[list(range(N_CORES))],
                ins=[cc1_in[:, :].opt()],
                outs=[cc1_out[:, :].opt()],
            )

            # ---------------- unpack: assemble full-seq K/Q/V/gates ----------
            # q/key column order: source-major (s*256 + local row), per batch.
            KT = []
            for b in range(B):
                ktile = gath.tile([64, N], BF, name=f"i{it}_KT{b}", tag=f"KTb{b}", bufs=1)
                src = cc1_out[:, OFF_K:OFF_K + 32768].rearrange(
                    "s (p f) -> p s f", p=64)[:, :, b * RB:(b + 1) * RB]
                nc.sync.dma_start(out=ktile.rearrange("p (s j) -> p s j", s=N_CORES), in_=src)
                KT.append(ktile)
            Qb = [[None, None], [None, None]]
            for g in range(2):
                for b in range(B):
                    qtile = gath.tile([64, N], BF, name=f"i{it}_Qb{g}_{b}", tag=f"Qb{g}{b}", bufs=1)
                    src = cc1_out[:, OFF_Q:OFF_Q + 65536].rearrange(
                        "s (p f) -> p s f", p=64)[:, :, g * R + b * RB: g * R + (b + 1) * RB]
                    nc.sync.dma_start(out=qtile.rearrange("p (s j) -> p s j", s=N_CORES), in_=src)
                    Qb[g][b] = qtile
            Vrow = []
            for j in range(4):
                vtile = gath.tile([128, N_CORES * 65], BF, name=f"i{it}_Vrow{j}", tag=f"Vrow{j}", bufs=1)
                src = cc1_out[:, OFF_V + j * VSLAB:OFF_V + (j + 1) * VSLAB].rearrange(
                    "s (p f) -> p s f", p=128)
                nc.sync.dma_start(out=vtile.rearrange("p (s e) -> p s e", s=N_CORES), in_=src)
                Vrow.append(vtile)
            # gates: partition (g*16 + b*8 + s)*... -> use stride-4 partitions
            # gates row (g,b) at partition 32*(2g+b); col slab vq*256
            gsp2 = consts.tile([128, N], BF, bufs=1, name=f"i{it}_gsp2", tag="gsp2")
            for g_ in range(2):
                for b_ in range(2):
                    pstart = 32 * (2 * g_ + b_)
                    nc.sync.dma_start(
                        out=gsp2[pstart:pstart + 1, :].rearrange("p (s j) -> p s j", s=N_CORES),
                        in_=cc1_out[:, OFF_G + g_ * 512 + b_ * RB:
                                    OFF_G + g_ * 512 + b_ * RB + RB].unsqueeze(0))

            # ---------------- v_hat (l2norm of V rows, transposed) -----------
            vhT = []
            for b in range(B):
                vhT.append(gath.tile([64, N], FP, name=f"i{it}_vhT{b}", tag=f"vhT{b}", bufs=1))
            for j in range(4):
                b = j // 2
                vd = Vrow[j].rearrange("p (s e) -> p s e", e=65)[:, :, 0:64]
                vsq = scr.tile([128, 512], FP, name=f"i{it}_vsq{j}", tag="SA", bufs=1)
                nc.vector.tensor_mul(vsq.rearrange("p (s d) -> p s d", d=DH), vd, vd)
                ssv = smalls.tile([128, N_CORES], FP, name=f"i{it}_ssv{j}", tag="ssv", bufs=2)
                nc.vector.tensor_reduce(out=ssv, in_=vsq.rearrange("p (s d) -> p s d", d=DH),
                                        axis=mybir.AxisListType.X, op=ADD)
                nc.scalar.activation(out=ssv, in_=ssv, func=Sqrt, bias=eps24[:, 0:1])
                nc.vector.reciprocal(out=ssv, in_=ssv)
                vh = scr.tile([128, 512], FP, name=f"i{it}_vh{j}", tag="SB", bufs=1)
                rv_b = ssv[:, :].unsqueeze(-1).to_broadcast([128, N_CORES, DH])
                nc.vector.tensor_mul(vh.rearrange("p (s d) -> p s d", d=DH), vd, rv_b)
                for pr in range(4):
                    tp = ps.tile([128, 128], FP, name=f"i{it}_vtp{j}_{pr}", tag="bsmall", bufs=2)
                    nc.tensor.transpose(tp, vh[:, pr * 128:(pr + 1) * 128], ident)
                    nc.vector.tensor_copy(
                        vhT[b][:, (2 * pr) * RB + (j % 2) * 128: (2 * pr) * RB + (j % 2) * 128 + 128],
                        tp[0:64, :])
                    nc.vector.tensor_copy(
                        vhT[b][:, (2 * pr + 1) * RB + (j % 2) * 128: (2 * pr + 1) * RB + (j % 2) * 128 + 128],
                        tp[64:128, :])

            # ---------------- attention + belief, head-sharded ---------------
            finT = [finpool.tile([128, N], BF, name=f"i{it}_finT{b}", tag=f"finT{b}", bufs=1)
                    for b in range(B)]
            cc2_ins = [dram.tile([N_CORES, 32768], BF, name=f"i{it}_cc2_in{b}", tag=f"cc2_in{b}", bufs=2)
                       for b in range(B)]
            cc2_outs = []
            for b in range(B):
                for vq in range(8):
                    qsl = slice(vq * RB, (vq + 1) * RB)
                    rhs_g = [Qb[0][b][:, qsl], Qb[1][b][:, qsl]]
                    avp = ps.tile([65, 512], FP, name=f"i{it}_avp{b}_{vq}", tag="avp", bufs=2)
                    for q4 in range(4):
                        ssb = simsb.tile([128, 2048], BF, name=f"i{it}_ssb{b}_{vq}_{q4}", tag="ssb", bufs=3)
                        for hf in range(2):
                            sp = ps.tile([128, 1024], FP, name=f"i{it}_sp{b}_{vq}_{q4}_{hf}",
                                         tag="acc", bufs=2)
                            for i in range(2):
                                m = q4 * 4 + hf * 2 + i
                                for g in range(2):
                                    MM(sp[:, i * 512 + g * 256: i * 512 + (g + 1) * 256],
                                       KT[b][:, m * 128:(m + 1) * 128],
                                       rhs_g[g], start=True, stop=True)
                            nc.scalar.activation(out=ssb[:, hf * 1024:(hf + 1) * 1024],
                                                 in_=sp, func=Exp, scale=SCALE)
                        for i in range(4):
                            m = q4 * 4 + i
                            MM(avp, Vrow[b * 2 + (m % 2)][:, (m // 2) * 65:(m // 2) * 65 + 65],
                                             ssb[:, i * 512:(i + 1) * 512],
                                             start=(m == 0), stop=(m == 15))
                    avsb = finpool.tile([65, 512], FP, name=f"i{it}_avsb{b}_{vq}", tag=f"avs{vq % 3}", bufs=1)
                    nc.vector.tensor_copy(avsb, avp)
                    _cache.setdefault("_avsb", {})[(it, b, vq)] = avsb

                # ---- belief + gating for this batch's q blocks ----
                for vq in range(8):
                    avsb = _cache["_avsb"][(it, b, vq)]
                    vhdup = vhT[b][:, vq * RB:(vq + 1) * RB].unsqueeze(1).to_broadcast([64, 2, RB])
                    prod = scr.tile([64, 512], FR, name=f"i{it}_prod{b}_{vq}", tag="SA", bufs=1)
                    nc.vector.tensor_mul(prod.rearrange("p (g r) -> p g r", r=RB),
                                         avsb[0:64, :].rearrange("p (g r) -> p g r", r=RB),
                                         vhdup)
                    dotp = ps.tile([1, 512], FP, name=f"i{it}_dotp{b}_{vq}", tag="bsmall", bufs=2)
                    MM(dotp, ones128r[0:64, 0:1], prod, start=True, stop=True)
                    dsb = smalls.tile([1, 512], FR, name=f"i{it}_dsb{b}_{vq}", tag="dsb", bufs=1)
                    nc.vector.tensor_copy(dsb, dotp)
                    rcp = smalls.tile([1, 512], FP, name=f"i{it}_rcp{b}_{vq}", tag="rcp", bufs=1)
                    nc.vector.reciprocal(out=rcp, in_=avsb[64:65, :])
                    gA = smalls.tile([1, 256], FP, name=f"i{it}_gA{b}_{vq}", tag="gA", bufs=1)
                    gB = smalls.tile([1, 256], FP, name=f"i{it}_gB{b}_{vq}", tag="gB", bufs=1)
                    pa = 32 * b
                    pb_ = 32 * (2 + b)
                    nc.vector.tensor_copy(gA, gsp2[pa:pa + 1, vq * RB:(vq + 1) * RB])
                    nc.vector.tensor_copy(gB, gsp2[pb_:pb_ + 1, vq * RB:(vq + 1) * RB])
                    scl = smalls.tile([1, 512], FR, name=f"i{it}_scl{b}_{vq}", tag="scl", bufs=1)
                    nc.vector.tensor_mul(scl[:, 0:RB], gA, rcp[:, 0:RB])
                    nc.vector.tensor_mul(scl[:, RB:2 * RB], gB, rcp[:, RB:2 * RB])
                    dotb = ps.tile([64, 512], FP, name=f"i{it}_dotb{b}_{vq}", tag="bsmall", bufs=2)
                    MM(dotb, ones1r, dsb, start=True, stop=True)
                    t1 = scr.tile([64, 512], FP, name=f"i{it}_t1_{b}_{vq}", tag="SB", bufs=1)
                    nc.vector.tensor_mul(t1.rearrange("p (g r) -> p g r", r=RB),
                                         dotb.rearrange("p (g r) -> p g r", r=RB), vhdup)
                    t2 = scr.tile([64, 512], FP, name=f"i{it}_t2_{b}_{vq}", tag="SA", bufs=1)
                    nc.vector.tensor_sub(t2, avsb[0:64, :], t1)
                    sclb = ps.tile([64, 512], FP, name=f"i{it}_sclb{b}_{vq}", tag="bsmall", bufs=2)
                    MM(sclb, ones1r, scl, start=True, stop=True)
                    nc.vector.tensor_mul(finT[b][0:64, vq * RB:(vq + 1) * RB], t2[:, 0:RB], sclb[:, 0:RB])
                    nc.vector.tensor_mul(finT[b][64:128, vq * RB:(vq + 1) * RB], t2[:, RB:2 * RB], sclb[:, RB:2 * RB])

                # ---- pack + AllToAll #2 for this batch (b0's overlaps b1) ----
                for d in range(N_CORES):
                    nc.sync.dma_start(
                        out=cc2_ins[b][d, :].rearrange("(p f) -> p f", p=128),
                        in_=finT[b][:, d * RB:(d + 1) * RB])
                cc2_out = dram.tile([N_CORES, 32768], BF, name=f"i{it}_cc2_out{b}", tag=f"cc2_out{b}", bufs=2)
                nc.gpsimd.collective_compute(
                    "AllToAll", mybir.AluOpType.bypass,
                    replica_groups=[list(range(N_CORES))],
                    ins=[cc2_ins[b][:, :].opt()],
                    outs=[cc2_out[:, :].opt()],
                )
                cc2_outs.append(cc2_out)

            # ---------------- output projection (bf16), per batch ------------
            fin = [finpool.tile([128, R], BF, name=f"i{it}_fin{s}", tag=f"fin{s}", bufs=1)
                   for s in range(N_CORES)]
            for s in range(N_CORES):
                for b in range(B):
                    nc.sync.dma_start(
                        out=fin[s][:, b * RB:(b + 1) * RB],
                        in_=cc2_outs[b][s, :].rearrange("(p f) -> p f", p=128))
            for b in range(B):
                wop = {}
                pA = ps.tile([128, 1024], FP, name=f"i{it}_woA{b}", tag="acc", bufs=2)
                pB = ps.tile([128, 1024], FP, name=f"i{it}_woB{b}", tag="acc", bufs=2)
                pC = ps.tile([128, 512], FP, name=f"i{it}_woC{b}", tag="avp", bufs=2)
                pD = ps.tile([128, 512], FP, name=f"i{it}_woD{b}", tag="avp", bufs=2)
                pE = ps.tile([128, 512], FP, name=f"i{it}_woE{b}", tag="bsmall", bufs=2)
                pF = ps.tile([128, 512], FP, name=f"i{it}_woF{b}", tag="bsmall", bufs=2)
                wop[(0, 0)] = pA[:, 0:512]
                wop[(0, 1)] = pA[:, 512:1024]
                wop[(0, 2)] = pB[:, 0:512]
                wop[(0, 3)] = pB[:, 512:1024]
                wop[(1, 0)] = pC
                wop[(1, 1)] = pD
                wop[(1, 2)] = pE
                wop[(1, 3)] = pF
                for kt in range(8):
                    for wh in range(2):
                        wos = wstream.tile([128, 1024], BF, name=f"i{it}_wos{b}_{kt}_{wh}", tag="wos", bufs=3)
                        nc.sync.dma_start(out=wos, in_=wo_in[kt * 128:(kt + 1) * 128,
                                                             wh * 1024:(wh + 1) * 1024])
                        for rt in range(2):
                            lhsT = fin[kt][:, b * RB + rt * 128: b * RB + (rt + 1) * 128]
                            for ch2 in range(2):
                                ch = wh * 2 + ch2
                                MM(wop[(rt, ch)], lhsT,
                                                 wos[:, ch2 * 512:(ch2 + 1) * 512],
                                                 start=(kt == 0), stop=(kt == 7))
                for rt in range(2):
                    for ch in range(4):
                        osb = scr.tile([128, 512], FP, name=f"i{it}_osb{b}_{rt}_{ch}", tag="SC", bufs=2)
                        nc.vector.tensor_copy(osb, wop[(rt, ch)])
                        nc.sync.dma_start(
                            out=out_dram[b * RB + rt * 128: b * RB + (rt + 1) * 128,
                                         ch * 512:(ch + 1) * 512],
                            in_=osb)

    _cache.pop("_avsb", None)
    nc.compile()
    return nc


def _prep_inputs(tokens, rotary_pos_emb, residual_values, rms_w, Wq, Wk, Wv, Wo, Wg, gamma, Wmix, bmix):
    tokens = np.asarray(tokens, np.float32)
    rot = np.asarray(rotary_pos_emb, np.float32)
    res = np.asarray(residual_values, np.float32)
    rms_w = np.asarray(rms_w, np.float32)
    Wq_ = np.ascontiguousarray(np.asarray(Wq, np.float32) * rms_w[:, None])
    Wk_ = np.ascontiguousarray(np.asarray(Wk, np.float32) * rms_w[:, None])
    Wv_ = np.asarray(Wv, np.float32) * rms_w[:, None]
    Wmix_ = np.asarray(Wmix, np.float32) * rms_w[:, None]
    Wg_ = np.asarray(Wg, np.float32) * rms_w[:, None]
    np_bf = mybir.dt.np(mybir.dt.bfloat16)
    Wo_ = np.ascontiguousarray(np.asarray(Wo, np.float32).astype(np_bf))
    bmix = np.asarray(bmix, np.float32)
    gamma = np.asarray(gamma, np.float32)

    wcat = np.ascontiguousarray(np.concatenate([Wv_, Wmix_, Wg_], axis=1))
    cos_full = np.cos(rot)
    sin_full = np.sin(rot)
    sign = np.where(np.arange(DH) < 32, -1.0, 1.0).astype(np.float32)

    gamma_sc = np.zeros((128, 4), np.float32)
    gamma_scp = np.zeros((128, 4), np.float32)
    gfull = (gamma + 1.0) * (DH ** 0.5)   # (8, 64)
    for t in range(4):
        for j in range(2):
            h = 2 * t + j
            gamma_sc[j * 64:(j + 1) * 64, t] = gfull[h]
            gamma_scp[j * 64:(j + 1) * 64, t] = gfull[h][np.arange(DH) ^ 32]
    bmix_t = np.broadcast_to(bmix[None, :], (128, H)).copy()
    ident = np.eye(128, dtype=np.float32)
    bd2 = np.zeros((128, 2), np.float32)
    bd2[0:64, 0] = 1.0
    bd2[64:128, 1] = 1.0
    bd2T = np.ascontiguousarray(bd2.T)
    ones128 = np.ones((128, 1), np.float32)
    ones1 = np.ones((1, 64), np.float32)

    in_maps = []
    for c in range(N_CORES):
        sl = slice(c * RB, (c + 1) * RB)
        x_loc = np.ascontiguousarray(np.concatenate([tokens[0, sl], tokens[1, sl]], axis=0))
        res_pp = np.zeros((4, 128, 512), np.float32)
        for b in range(B):
            for jt in range(2):
                blk = res[b, :, c * RB + jt * 128: c * RB + (jt + 1) * 128, :]
                res_pp[b * 2 + jt] = blk.transpose(1, 0, 2).reshape(128, 512)
        cosT = np.zeros((128, R), np.float32)
        sinTs = np.zeros((128, R), np.float32)
        cs = cos_full[sl].T   # (64, 256)
        sn = sin_full[sl].T * sign[:, None]
        for b in range(B):
            cosT[0:64, b * RB:(b + 1) * RB] = cs
            cosT[64:128, b * RB:(b + 1) * RB] = cs
            sinTs[0:64, b * RB:(b + 1) * RB] = sn
            sinTs[64:128, b * RB:(b + 1) * RB] = sn
        in_maps.append({
            "x_loc": x_loc,
            "wq": Wq_, "wk": Wk_, "wcat": wcat, "wo_bf": Wo_,
            "res_pp": res_pp,
            "cosT": cosT, "sinTs": sinTs,
            "gamma_sc": gamma_sc, "gamma_scp": gamma_scp, "bmix_t": bmix_t,
            "ident": ident, "bd2": bd2, "bd2T": bd2T,
            "ones128": ones128, "ones128r": np.ones((128, 32), np.float32), "ones1": ones1, "ones1r": ones1,
        })
    return in_maps


def kernel(**inputs):
    if "nc" not in _cache:
        _cache["nc"] = build_kernel()
    nc = _cache["nc"]
    in_maps = _prep_inputs(**inputs)
    trace = os.environ.get("KTRACE", "0") == "1"
    res = run_bass_kernel_spmd(nc, in_maps, core_ids=list(range(N_CORES)), trace=trace)
    _cache["last_result"] = res
    out = np.zeros((B, N, D), np.float32)
    for c in range(N_CORES):
        o = res.results[c]["out_loc"]
        sl = slice(c * RB, (c + 1) * RB)
        out[0, sl] = o[0:RB]
        out[1, sl] = o[RB:2 * RB]
    return out
